# revision 9
# baseline (speedup 1.0000x reference)
"""GRU-D decoder kernel for Trainium2 (8 NeuronCores, data-parallel over batch).

Math (mask == ones everywhere, which the reference hardcodes):
  x_hat = C (constant), d = dt broadcast, gamma_x unused.
  gamma[t,b,j] = exp(-relu(dt[t,b] * colsum(Wgh)[j] + bgh[j]))   (precomputed host-side)
  per step: hdec = gamma_t * h
            z = sigmoid(hdec @ Wz_h + Az0);  r = sigmoid(hdec @ Wr_h + Ar0)
            htl = tanh((r*hdec) @ Wh_h + Ah0)
            h = hdec + z*(htl - hdec)
  out[t] = h_t @ Wlin            (blin added host-side after the gather)
  where A?0 = C @ W?_x + colsum(W?_m) + b?  (time-constant, precomputed host-side).

Device layout: everything transposed (H on partitions as 4 tiles of 128,
batch=64 on the free dim), packed as SBUF tiles (128, 4*64) with column
index = kt*64 + b.  State is bf16; gate matmul operands are fp8e4m3
scaled by 16 (validated host-side: global rel err ~7.3e-3 vs 2e-2 gate).

v4 structure:
  - Gate matmuls run fp8 DoubleRow: 2 fp8 weights per PE cell -> K=256 per
    instruction, so each gate is 8 matmuls instead of 16.  24 weight-tile
    switches per step instead of 48 (LDWEIGHTS is the serial bottleneck).
  - Per-step PE stream r(8) z(8) htl(8) proj(2) inits(4); the half-pair
    projection + next-step psum inits fill the tanh/blend tail every step
    (keeps the HAM clock gate at 2.4 GHz).
  - Projection batches two timesteps per weight pass from a bf16 h ring
    (M=128, N=512), split across the pair's two steps.
"""

import numpy as np
import ml_dtypes

T, B, H, O = 100, 512, 512, 512
NCORES = 8
BL = B // NCORES  # 64
KC = 4  # contraction chunks of 128
KC2 = 2  # DoubleRow contraction chunks of 256
JT = 4  # output j-tiles of 128
FR = JT * BL  # 256
HB = FR // 2  # 128 (half of the free dim; = 2 j-tiles)
GCH = 20  # gamma chunk (steps per DMA)
PSB = 512  # psum bank width in fp32
WSCL = 16.0  # fp8 weight scale (psum carries 16x the preactivation)

_BUILD_CACHE = {}


def _build_program():
    if "nc" in _BUILD_CACHE:
        return _BUILD_CACHE["nc"]

    import concourse.tile as tile
    import concourse.mybir as mybir
    from concourse import bacc
    from contextlib import ExitStack

    f32 = mybir.dt.float32
    bf16 = mybir.dt.bfloat16
    fp8 = mybir.dt.float8e4
    AF = mybir.ActivationFunctionType
    DR = mybir.MatmulPerfMode.DoubleRow

    nc = bacc.Bacc("TRN2", target_bir_lowering=False, debug=False,
                   num_devices=NCORES)

    gam_d = nc.dram_tensor("gam", [128, T, FR], bf16, kind="ExternalInput")
    wzr_d = nc.dram_tensor("wzr", [128, KC2 * 2 * JT * 2, 128], fp8, kind="ExternalInput")
    wht_d = nc.dram_tensor("wht", [128, KC2 * JT * 2, 128], fp8, kind="ExternalInput")
    wlin_d = nc.dram_tensor("wlin", [128, KC * O], bf16, kind="ExternalInput")
    a0z_d = nc.dram_tensor("a0z", [128, FR], bf16, kind="ExternalInput")
    a0r_d = nc.dram_tensor("a0r", [128, FR], bf16, kind="ExternalInput")
    a0h_d = nc.dram_tensor("a0h", [128, FR], bf16, kind="ExternalInput")
    ident_d = nc.dram_tensor("ident", [128, 128], bf16, kind="ExternalInput")
    out_d = nc.dram_tensor("out", [T, BL, O], f32, kind="ExternalOutput")

    with tile.TileContext(nc) as tc, ExitStack() as ctx:
        constp = ctx.enter_context(tc.tile_pool(name="const", bufs=1))
        gpool = ctx.enter_context(tc.tile_pool(name="gam", bufs=2))
        ringp = ctx.enter_context(tc.tile_pool(name="ring", bufs=1))
        hdp = ctx.enter_context(tc.tile_pool(name="hd", bufs=2))
        actp = ctx.enter_context(tc.tile_pool(name="act", bufs=2))
        osbp = ctx.enter_context(tc.tile_pool(name="osb", bufs=2))
        prp = ctx.enter_context(tc.tile_pool(name="pr", bufs=1, space="PSUM"))
        pzp = ctx.enter_context(tc.tile_pool(name="pz", bufs=1, space="PSUM"))
        php0 = ctx.enter_context(tc.tile_pool(name="ph0", bufs=2, space="PSUM"))
        php1 = ctx.enter_context(tc.tile_pool(name="ph1", bufs=2, space="PSUM"))
        pjp = ctx.enter_context(tc.tile_pool(name="pj", bufs=2, space="PSUM"))

        wzr = constp.tile([128, KC2 * 2 * JT * 2, 128], fp8)
        nc.sync.dma_start(wzr[:], wzr_d[:])
        wht = constp.tile([128, KC2 * JT * 2, 128], fp8)
        nc.sync.dma_start(wht[:], wht_d[:])
        wlin = constp.tile([128, KC * O], bf16)
        nc.sync.dma_start(wlin[:], wlin_d[:])
        a0z = constp.tile([128, FR], bf16)
        nc.sync.dma_start(a0z[:], a0z_d[:])
        a0r = constp.tile([128, FR], bf16)
        nc.sync.dma_start(a0r[:], a0r_d[:])
        a0h = constp.tile([128, FR], bf16)
        nc.sync.dma_start(a0h[:], a0h_d[:])
        ident = constp.tile([128, 128], bf16)
        nc.sync.dma_start(ident[:], ident_d[:])

        # h ring buffer, bf16: column = (kt, slot*BL + b) with slot = t%4, so
        # a projection pair (t, t+1) is a contiguous 128-column slice per kt
        # block (walrus requires 2D stationary APs).  Elementwise ops use 3D
        # strided views (two kt blocks per half).
        hring = ringp.tile([128, KC, 4 * BL], bf16)

        def ring_blk(kt, s, n=1):
            return hring[:, kt, s * BL:(s + n) * BL]

        def ring_half(hf, s):
            return hring[:, 2 * hf:2 * hf + 2, s * BL:(s + 1) * BL]

        def wzr_blk(g, jo, kc2):
            i = ((kc2 * 2 + g) * JT + jo) * 2
            return wzr[:, i:i + 2, :]

        def wht_blk(jo, kc2):
            i = (kc2 * JT + jo) * 2
            return wht[:, i:i + 2, :]

        # gamma chunks, preloaded half a chunk ahead
        chunks = {}

        def ensure_chunk(c):
            if c in chunks or c * GCH >= T:
                return
            t0 = c * GCH
            t1 = min(t0 + GCH, T)
            gt = gpool.tile([128, GCH * FR], bf16, tag="gchunk")
            nc.sync.dma_start(gt[:, 0:(t1 - t0) * FR], gam_d[:, t0:t1, :])
            chunks[c] = gt

        ensure_chunk(0)

        # step-0 decayed state is zero.  hd8 is the fp8 matmul operand (3D
        # view: [128, kt, b]); hdb the bf16 copy used by the blend.
        hd8 = hdp.tile([128, KC, BL], fp8, tag="hd8")
        nc.vector.memset(hd8[:], 0.0)
        hdb = hdp.tile([128, FR], bf16, tag="hdb")
        nc.vector.memset(hdb[:], 0.0)

        def make_inits():
            """Allocate next step's psum tiles and preload the (16x-scaled)
            gate constants via identity matmuls at the end of the PE stream."""
            pr = prp.tile([128, PSB], f32, tag="pr")
            nc.tensor.matmul(pr[:, 0:FR], ident[:], a0r[:], start=True, stop=False)
            pz = pzp.tile([128, PSB], f32, tag="pz")
            nc.tensor.matmul(pz[:, 0:FR], ident[:], a0z[:], start=True, stop=False)
            ph0 = php0.tile([128, PSB], f32, tag="ph0")
            nc.tensor.matmul(ph0[:, 0:HB], ident[:], a0h[:, 0:HB], start=True, stop=False)
            ph1 = php1.tile([128, PSB], f32, tag="ph1")
            nc.tensor.matmul(ph1[:, 0:HB], ident[:], a0h[:, HB:FR], start=True, stop=False)
            return pr, pz, ph0, ph1

        def issue_proj(t0, pj, kcs):
            """Project the h pair (t0, t0+1) from the ring: accumulating
            matmuls with M=128 (two steps x 64 batch), N=512.  Split across
            two scan steps (kcs=(0,1) then (2,3)) so both steps' PE tails
            get fill work."""
            base = t0 % 4
            for kc in kcs:
                nc.tensor.matmul(
                    pj[:],
                    ring_blk(kc, base, 2),
                    wlin[:, kc * O:(kc + 1) * O],
                    start=(kc == 0), stop=(kc == KC - 1),
                )

        def evac_proj(t0, pj):
            osb = osbp.tile([128, O], f32, tag="osb")
            nc.scalar.copy(osb[:, 0:256], pj[:, 0:256])
            nc.vector.tensor_copy(osb[:, 256:512], pj[:, 256:512])
            nc.sync.dma_start(out_d[t0:t0 + 2], osb[:])

        pr, pz, ph0, ph1 = make_inits()
        pj_cur = None

        for t in range(T):
            c, o = divmod(t, GCH)
            if o == GCH // 2:
                ensure_chunk(c + 1)
            slot = t % 4

            # ---- r gate matmuls (DoubleRow, K=256), kc2-outer so they start
            # on the partial hd of the previous step's first blend half
            for kc2 in range(KC2):
                for jo in range(JT):
                    nc.tensor.matmul(
                        pr[:, jo * BL:(jo + 1) * BL],
                        wzr_blk(1, jo, kc2),
                        hd8[:, kc2 * 2:kc2 * 2 + 2, :],
                        start=False, stop=(kc2 == KC2 - 1), perf_mode=DR,
                    )
            # ---- z gate matmuls (fill the sigmoid(r)/rh window)
            for kc2 in range(KC2):
                for jo in range(JT):
                    nc.tensor.matmul(
                        pz[:, jo * BL:(jo + 1) * BL],
                        wzr_blk(0, jo, kc2),
                        hd8[:, kc2 * 2:kc2 * 2 + 2, :],
                        start=False, stop=(kc2 == KC2 - 1), perf_mode=DR,
                    )
            rb = actp.tile([128, FR], bf16, tag="rb")
            nc.scalar.activation(rb[:], pr[:, 0:FR], AF.Sigmoid, scale=1.0 / WSCL)
            rh8 = hdp.tile([128, KC, BL], fp8, tag="rh8")
            nc.vector.tensor_mul(rh8[:], rb[:], hdb[:])

            # ---- candidate gate, jo-major: ph0 (h-half 0) completes first so
            # tanh(half 0) overlaps the jo 2,3 accumulation
            for jo in range(JT):
                tgt, col = (ph0, jo) if jo < 2 else (ph1, jo - 2)
                for kc2 in range(KC2):
                    nc.tensor.matmul(
                        tgt[:, col * BL:(col + 1) * BL],
                        wht_blk(jo, kc2),
                        rh8[:, kc2 * 2:kc2 * 2 + 2, :],
                        start=False, stop=(kc2 == KC2 - 1), perf_mode=DR,
                    )
            zf = actp.tile([128, FR], bf16, tag="zf")
            nc.scalar.activation(zf[:], pz[:, 0:FR], AF.Sigmoid, scale=1.0 / WSCL)

            # ---- tail fill on PE: half a pair-projection every step
            if t >= 2 and t % 2 == 0:
                pj_cur = pjp.tile([128, PSB], f32, tag="pj")
                issue_proj(t - 2, pj_cur, (0, 1))
            elif t >= 3 and t % 2 == 1:
                issue_proj(t - 3, pj_cur, (2, 3))
            ph0_r, ph1_r = ph0, ph1
            if t + 1 < T:
                pr, pz, ph0, ph1 = make_inits()

            # ---- tanh + blend (h = hd + z*(htl-hd)), then decay for t+1
            hd8_n = hdb_n = None
            if t + 1 < T:
                hd8_n = hdp.tile([128, KC, BL], fp8, tag="hd8")
                hdb_n = hdp.tile([128, FR], bf16, tag="hdb")
            for hf, ph in ((0, ph0_r), (1, ph1_r)):
                sl = slice(hf * HB, (hf + 1) * HB)
                htl = actp.tile([128, HB], bf16, tag=f"htl{hf}")
                nc.scalar.activation(htl[:], ph[:, 0:HB], AF.Tanh, scale=1.0 / WSCL)
                dd = actp.tile([128, HB], bf16, tag=f"dd{hf}")
                nc.vector.tensor_sub(dd[:], htl[:], hdb[:, sl])
                ee = actp.tile([128, HB], bf16, tag=f"ee{hf}")
                nc.vector.tensor_mul(ee[:], zf[:, sl], dd[:])
                nc.vector.tensor_add(ring_half(hf, slot), hdb[:, sl], ee[:])
                if t + 1 < T:
                    gsl = chunks[(t + 1) // GCH][
                        :, ((t + 1) % GCH) * FR + hf * HB:
                           ((t + 1) % GCH) * FR + (hf + 1) * HB]
                    nc.vector.tensor_mul(hd8_n[:, 2 * hf:2 * hf + 2, :], gsl,
                                         ring_half(hf, slot))
                    nc.vector.tensor_mul(hdb_n[:, sl], gsl, ring_half(hf, slot))
            if t + 1 < T:
                hd8, hdb = hd8_n, hdb_n

            # ---- drain the finished projection pair (psum -> sbuf -> HBM)
            if t >= 3 and t % 2 == 1:
                evac_proj(t - 3, pj_cur)

        # final pair (T-2, T-1)
        pj_cur = pjp.tile([128, PSB], f32, tag="pj")
        issue_proj(T - 2, pj_cur, (0, 1, 2, 3))
        evac_proj(T - 2, pj_cur)

    nc.compile()
    _BUILD_CACHE["nc"] = nc
    return nc


def _host_prep(C, t, Wz, bz, Wr, br, Wh, bh, Wgh, bgh, Wlin):
    """Build per-core input maps (all the precomputed, packed device tensors)."""
    bf = ml_dtypes.bfloat16
    f8 = ml_dtypes.float8_e4m3

    s = Wgh.sum(axis=0)  # (H,)
    t3 = t[:, :, 0]  # (T,B)
    dt = np.concatenate([np.zeros((1, B), np.float32), t3[1:] - t3[:-1]], axis=0)
    # gamma (T,B,H)
    gam = np.exp(-np.maximum(dt[:, :, None] * s[None, None, :] + bgh[None, None, :], 0.0)).astype(np.float32)

    def gate_const(W, b):
        # 16 * (C @ W_x + colsum(W_m) + b)  -> (B,H); matches the 16x-scaled
        # fp8 gate weights, undone by the activation scale.
        return WSCL * (C @ W[0:H] + (W[2 * H:3 * H].sum(axis=0) + b)[None, :])

    Az0 = gate_const(Wz, bz).astype(np.float32)
    Ar0 = gate_const(Wr, br).astype(np.float32)
    Ah0 = gate_const(Wh, bh).astype(np.float32)

    # DoubleRow packing: [k, (kc2, g, jo, ko), m] with input row = kc2*256 +
    # ko*128 + k and output col = jo*128 + m.
    Wg = np.stack([Wz[H:2 * H], Wr[H:2 * H]]) * WSCL  # (2,H,H)
    wzr = Wg.reshape(2, KC2, 2, 128, JT, 128).transpose(3, 1, 0, 4, 2, 5) \
            .reshape(128, KC2 * 2 * JT * 2, 128)
    whs = (Wh[H:2 * H] * WSCL).reshape(KC2, 2, 128, JT, 128).transpose(2, 0, 3, 1, 4) \
            .reshape(128, KC2 * JT * 2, 128)
    wlin = Wlin.reshape(KC, 128, O).transpose(1, 0, 2).reshape(128, KC * O)
    wzr = np.ascontiguousarray(wzr, dtype=f8)
    whs = np.ascontiguousarray(whs, dtype=f8)
    wlin = np.ascontiguousarray(wlin, dtype=bf)
    ident = np.eye(128, dtype=bf)

    in_maps = []
    for i in range(NCORES):
        sl = slice(i * BL, (i + 1) * BL)
        gf = gam[:, sl, :]  # (T,BL,H)
        # gam packed: [p, t, kt*BL+b]
        gp = np.ascontiguousarray(
            gf.reshape(T, BL, KC, 128).transpose(3, 0, 2, 1).reshape(128, T, KC * BL),
            dtype=bf)

        def packA(A):
            return np.ascontiguousarray(
                A[sl].reshape(BL, JT, 128).transpose(2, 1, 0).reshape(128, JT * BL), dtype=bf)

        in_maps.append({
            "gam": gp,
            "wzr": wzr,
            "wht": whs,
            "wlin": wlin,
            "a0z": packA(Az0),
            "a0r": packA(Ar0),
            "a0h": packA(Ah0),
            "ident": ident,
        })
    return in_maps


def kernel(C, t, mask, Wz, bz, Wr, br, Wh, bh, Wgh, bgh, wgx, bgx, Wlin, blin,
           _trace=False, _trace_kwargs=None):
    C = np.asarray(C, np.float32)
    t = np.asarray(t, np.float32)
    nc = _build_program()
    in_maps = _host_prep(C, t,
                         np.asarray(Wz, np.float32), np.asarray(bz, np.float32),
                         np.asarray(Wr, np.float32), np.asarray(br, np.float32),
                         np.asarray(Wh, np.float32), np.asarray(bh, np.float32),
                         np.asarray(Wgh, np.float32), np.asarray(bgh, np.float32),
                         np.asarray(Wlin, np.float32))

    from concourse.bass_utils import run_bass_kernel_spmd
    res = run_bass_kernel_spmd(nc, in_maps, list(range(NCORES)),
                               trace=_trace, **(_trace_kwargs or {}))
    outs = [res.results[i]["out"] for i in range(NCORES)]
    full = np.concatenate(outs, axis=1).astype(np.float32)  # (T,B,O)
    full += np.asarray(blin, np.float32)[None, None, :]
    kernel._last_results = res
    return full


# revision 13
# speedup vs baseline: 1.3053x; 1.3053x over previous
"""GRU-D decoder kernel for Trainium2 (8 NeuronCores, data-parallel over batch).

Math (mask == ones everywhere, which the reference hardcodes):
  x_hat = C (constant), d = dt broadcast, gamma_x unused.
  gamma[t,b,j] = exp(-relu(dt[t,b] * colsum(Wgh)[j] + bgh[j]))   (precomputed host-side)
  per step: hdec = gamma_t * h
            z = sigmoid(hdec @ Wz_h + Az0);  r = sigmoid(hdec @ Wr_h + Ar0)
            htl = tanh((r*hdec) @ Wh_h + Ah0)
            h = hdec + z*(htl - hdec)
  out[t] = h_t @ Wlin            (blin added host-side after the gather)
  where A?0 = C @ W?_x + colsum(W?_m) + b?  (time-constant, precomputed host-side).

Device layout: everything transposed (H on partitions as 4 tiles of 128,
batch=64 on the free dim), packed as SBUF tiles (128, 4*64) with column
index = kt*64 + b.  All state is bf16 (validated: global rel err ~5e-3).

v2 structure (vs the v1 baseline):
  - Per-step PE stream is r(16) z(16) htl(16, jo-major) proj(4, even steps)
    next-step psum inits(4).  The projection + inits fill the tanh/blend
    tail so the PE never idles long enough for the HAM clock gate to
    re-throttle (v1 oscillated 1.2<->2.4 GHz the whole run).
  - Projection batches TWO timesteps per weight pass: lhsT = h ring slots
    (t, t+1) giving M=128, rhs = Wlin tiles at N=512.  5 MMs/step -> 2.
  - All gate activations output bf16; the h state is a bf16 ring buffer
    (4 slots) read directly as the projection's stationary operand, so the
    v1 per-step fp32 state + hbf copy + separate osb copy disappear.
  - ph0/ph1 psum pools are double-buffered so next-step inits never wait
    on the current tanh reads.
"""

import numpy as np
import ml_dtypes

T, B, H, O = 100, 512, 512, 512
NCORES = 8
BL = B // NCORES  # 64
KC = 4  # contraction chunks of 128
JT = 4  # output j-tiles of 128
FR = JT * BL  # 256
HB = FR // 2  # 128 (half of the free dim; = 2 j-tiles)
GCH = 20  # gamma chunk (steps per DMA)
PSB = 512  # psum bank width in fp32

_BUILD_CACHE = {}


def _build_program():
    if "nc" in _BUILD_CACHE:
        return _BUILD_CACHE["nc"]

    import concourse.tile as tile
    import concourse.mybir as mybir
    from concourse import bacc
    from contextlib import ExitStack

    f32 = mybir.dt.float32
    bf16 = mybir.dt.bfloat16
    AF = mybir.ActivationFunctionType

    nc = bacc.Bacc("TRN2", target_bir_lowering=False, debug=False,
                   num_devices=NCORES)

    gam_d = nc.dram_tensor("gam", [128, T, FR], bf16, kind="ExternalInput")
    wzr_d = nc.dram_tensor("wzr", [128, KC * 2 * JT * 128], bf16, kind="ExternalInput")
    wht_d = nc.dram_tensor("wht", [128, KC * JT * 128], bf16, kind="ExternalInput")
    wlin_d = nc.dram_tensor("wlin", [128, KC * O], bf16, kind="ExternalInput")
    a0z_d = nc.dram_tensor("a0z", [128, FR], bf16, kind="ExternalInput")
    a0r_d = nc.dram_tensor("a0r", [128, FR], bf16, kind="ExternalInput")
    a0h_d = nc.dram_tensor("a0h", [128, FR], bf16, kind="ExternalInput")
    ident_d = nc.dram_tensor("ident", [128, 128], bf16, kind="ExternalInput")
    out_d = nc.dram_tensor("out", [T, BL, O], f32, kind="ExternalOutput")

    with tile.TileContext(nc) as tc, ExitStack() as ctx:
        constp = ctx.enter_context(tc.tile_pool(name="const", bufs=1))
        gpool = ctx.enter_context(tc.tile_pool(name="gam", bufs=2))
        ringp = ctx.enter_context(tc.tile_pool(name="ring", bufs=1))
        hdp = ctx.enter_context(tc.tile_pool(name="hd", bufs=2))
        actp = ctx.enter_context(tc.tile_pool(name="act", bufs=2))
        osbp = ctx.enter_context(tc.tile_pool(name="osb", bufs=2))
        prp = ctx.enter_context(tc.tile_pool(name="pr", bufs=1, space="PSUM"))
        pzp = ctx.enter_context(tc.tile_pool(name="pz", bufs=1, space="PSUM"))
        php0 = ctx.enter_context(tc.tile_pool(name="ph0", bufs=2, space="PSUM"))
        php1 = ctx.enter_context(tc.tile_pool(name="ph1", bufs=2, space="PSUM"))
        pjp = ctx.enter_context(tc.tile_pool(name="pj", bufs=2, space="PSUM"))

        wzr = constp.tile([128, KC * 2 * JT * 128], bf16)
        nc.sync.dma_start(wzr[:], wzr_d[:])
        wht = constp.tile([128, KC * JT * 128], bf16)
        nc.sync.dma_start(wht[:], wht_d[:])
        wlin = constp.tile([128, KC * O], bf16)
        nc.sync.dma_start(wlin[:], wlin_d[:])
        a0z = constp.tile([128, FR], bf16)
        nc.sync.dma_start(a0z[:], a0z_d[:])
        a0r = constp.tile([128, FR], bf16)
        nc.sync.dma_start(a0r[:], a0r_d[:])
        a0h = constp.tile([128, FR], bf16)
        nc.sync.dma_start(a0h[:], a0h_d[:])
        ident = constp.tile([128, 128], bf16)
        nc.sync.dma_start(ident[:], ident_d[:])

        # h ring buffer, bf16: column = (kt, slot*BL + b) with slot = t%4, so
        # a projection pair (t, t+1) is a contiguous 128-column slice per kt
        # block (walrus requires 2D stationary APs).  Elementwise ops use 3D
        # strided views (two kt blocks per half).
        hring = ringp.tile([128, KC, 4 * BL], bf16)

        def ring_blk(kt, s, n=1):
            return hring[:, kt, s * BL:(s + n) * BL]

        def ring_half(hf, s):
            return hring[:, 2 * hf:2 * hf + 2, s * BL:(s + 1) * BL]

        def wzr_blk(g, jo, kc):
            i = ((kc * 2 + g) * JT + jo) * 128
            return wzr[:, i:i + 128]

        def wht_blk(jo, kc):
            i = (kc * JT + jo) * 128
            return wht[:, i:i + 128]

        # gamma chunks, preloaded half a chunk ahead
        chunks = {}

        def ensure_chunk(c):
            if c in chunks or c * GCH >= T:
                return
            t0 = c * GCH
            t1 = min(t0 + GCH, T)
            gt = gpool.tile([128, GCH * FR], bf16, tag="gchunk")
            nc.sync.dma_start(gt[:, 0:(t1 - t0) * FR], gam_d[:, t0:t1, :])
            chunks[c] = gt

        def gamma_half(tt, hf):
            c2, o2 = divmod(tt, GCH)
            return chunks[c2][:, o2 * FR + hf * HB: o2 * FR + (hf + 1) * HB]

        ensure_chunk(0)

        # step-0 decayed state is zero
        hd = hdp.tile([128, FR], bf16, tag="hd")
        nc.vector.memset(hd[:], 0.0)

        def make_inits():
            """Allocate next step's psum tiles and preload the gate constants
            (identity matmuls run at the end of the previous PE stream)."""
            pr = prp.tile([128, PSB], f32, tag="pr")
            nc.tensor.matmul(pr[:, 0:FR], ident[:], a0r[:], start=True, stop=False)
            pz = pzp.tile([128, PSB], f32, tag="pz")
            nc.tensor.matmul(pz[:, 0:FR], ident[:], a0z[:], start=True, stop=False)
            ph0 = php0.tile([128, PSB], f32, tag="ph0")
            nc.tensor.matmul(ph0[:, 0:HB], ident[:], a0h[:, 0:HB], start=True, stop=False)
            ph1 = php1.tile([128, PSB], f32, tag="ph1")
            nc.tensor.matmul(ph1[:, 0:HB], ident[:], a0h[:, HB:FR], start=True, stop=False)
            return pr, pz, ph0, ph1

        def issue_proj(t0, pj, kcs):
            """Project the h pair (t0, t0+1) from the ring: accumulating
            matmuls with M=128 (two steps x 64 batch), N=512.  Split across
            two scan steps (kcs=(0,1) then (2,3)) so both steps' PE tails
            get fill work."""
            base = t0 % 4
            for kc in kcs:
                nc.tensor.matmul(
                    pj[:],
                    ring_blk(kc, base, 2),
                    wlin[:, kc * O:(kc + 1) * O],
                    start=(kc == 0), stop=(kc == KC - 1),
                )

        def evac_proj(t0, pj):
            osb = osbp.tile([128, O], f32, tag="osb")
            nc.scalar.copy(osb[:, 0:256], pj[:, 0:256])
            nc.vector.tensor_copy(osb[:, 256:512], pj[:, 256:512])
            nc.sync.dma_start(out_d[t0:t0 + 2], osb[:])

        pr, pz, ph0, ph1 = make_inits()
        pj_cur = None
        evac_pending = None

        for t in range(T):
            c, o = divmod(t, GCH)
            if o == GCH // 2:
                ensure_chunk(c + 1)
            slot = t % 4

            # ---- r gate matmuls, kc-outer so they start on partial hd
            for kc in range(KC):
                for jo in range(JT):
                    nc.tensor.matmul(
                        pr[:, jo * BL:(jo + 1) * BL],
                        wzr_blk(1, jo, kc),
                        hd[:, kc * BL:(kc + 1) * BL],
                        start=False, stop=(kc == KC - 1),
                    )
            # ---- z gate matmuls (fill the sigmoid(r)/rh window)
            for kc in range(KC):
                for jo in range(JT):
                    nc.tensor.matmul(
                        pz[:, jo * BL:(jo + 1) * BL],
                        wzr_blk(0, jo, kc),
                        hd[:, kc * BL:(kc + 1) * BL],
                        start=False, stop=(kc == KC - 1),
                    )
            rb = actp.tile([128, FR], bf16, tag="rb")
            nc.scalar.activation(rb[:], pr[:, 0:FR], AF.Sigmoid)
            rh = hdp.tile([128, FR], bf16, tag="rh")
            nc.vector.tensor_mul(rh[:], rb[:], hd[:])

            # ---- drain last step's finished projection pair here: the ACT /
            # DVE copies land in the z-matmul window instead of queueing ahead
            # of the next sigmoid(r)/rh (which would stall the PE stream)
            if evac_pending is not None:
                evac_proj(*evac_pending)
                evac_pending = None

            # ---- candidate gate, jo-major: ph0 (h-half 0) completes first so
            # tanh(half 0) overlaps the jo 2,3 accumulation
            for jo in range(JT):
                tgt, col = (ph0, jo) if jo < 2 else (ph1, jo - 2)
                for kc in range(KC):
                    nc.tensor.matmul(
                        tgt[:, col * BL:(col + 1) * BL],
                        wht_blk(jo, kc),
                        rh[:, kc * BL:(kc + 1) * BL],
                        start=False, stop=(kc == KC - 1),
                    )
            zf = actp.tile([128, FR], bf16, tag="zf")
            nc.scalar.activation(zf[:], pz[:, 0:FR], AF.Sigmoid)

            # ---- tail fill on PE: half a pair-projection every step
            if t >= 2 and t % 2 == 0:
                pj_cur = pjp.tile([128, PSB], f32, tag="pj")
                issue_proj(t - 2, pj_cur, (0, 1))
            elif t >= 3 and t % 2 == 1:
                issue_proj(t - 3, pj_cur, (2, 3))
            ph0_r, ph1_r = ph0, ph1
            if t + 1 < T:
                pr, pz, ph0, ph1 = make_inits()

            # ---- tanh + blend (h = hd + z*(htl-hd)), then decay for t+1
            hd_n = None
            if t + 1 < T:
                hd_n = hdp.tile([128, FR], bf16, tag="hd")
            for hf, ph in ((0, ph0_r), (1, ph1_r)):
                sl = slice(hf * HB, (hf + 1) * HB)
                htl = actp.tile([128, HB], bf16, tag=f"htl{hf}")
                nc.scalar.activation(htl[:], ph[:, 0:HB], AF.Tanh)
                dd = actp.tile([128, HB], bf16, tag=f"dd{hf}")
                nc.vector.tensor_sub(dd[:], htl[:], hd[:, sl])
                ee = actp.tile([128, HB], bf16, tag=f"ee{hf}")
                nc.vector.tensor_mul(ee[:], zf[:, sl], dd[:])
                nc.vector.tensor_add(ring_half(hf, slot), hd[:, sl], ee[:])
                if t + 1 < T:
                    nc.vector.tensor_mul(
                        hd_n[:, sl],
                        chunks[(t + 1) // GCH][
                            :, ((t + 1) % GCH) * FR + hf * HB:
                               ((t + 1) % GCH) * FR + (hf + 1) * HB],
                        ring_half(hf, slot))
            if t + 1 < T:
                hd = hd_n

            # ---- mark the finished projection pair for draining next step
            if t >= 3 and t % 2 == 1:
                evac_pending = (t - 3, pj_cur)

        if evac_pending is not None:
            evac_proj(*evac_pending)
        # final pair (T-2, T-1)
        pj_cur = pjp.tile([128, PSB], f32, tag="pj")
        issue_proj(T - 2, pj_cur, (0, 1, 2, 3))
        evac_proj(T - 2, pj_cur)

    nc.compile()
    _BUILD_CACHE["nc"] = nc
    return nc


def _host_prep(C, t, Wz, bz, Wr, br, Wh, bh, Wgh, bgh, Wlin):
    """Build per-core input maps (all the precomputed, packed device tensors)."""
    bf = ml_dtypes.bfloat16

    s = Wgh.sum(axis=0)  # (H,)
    t3 = t[:, :, 0]  # (T,B)
    dt = np.concatenate([np.zeros((1, B), np.float32), t3[1:] - t3[:-1]], axis=0)
    # gamma (T,B,H)
    gam = np.exp(-np.maximum(dt[:, :, None] * s[None, None, :] + bgh[None, None, :], 0.0)).astype(np.float32)

    def gate_const(W, b):
        # C @ W_x + colsum(W_m) + b  -> (B,H)
        return C @ W[0:H] + (W[2 * H:3 * H].sum(axis=0) + b)[None, :]

    Az0 = gate_const(Wz, bz).astype(np.float32)
    Ar0 = gate_const(Wr, br).astype(np.float32)
    Ah0 = gate_const(Wh, bh).astype(np.float32)

    Wg = np.stack([Wz[H:2 * H], Wr[H:2 * H]])  # (2,H,H)
    # wzr packed: [k, (kc,g,jo,m)]
    wzr = Wg.reshape(2, KC, 128, JT, 128).transpose(2, 1, 0, 3, 4).reshape(128, KC * 2 * JT * 128)
    wht = Wh[H:2 * H].reshape(KC, 128, JT, 128).transpose(1, 0, 2, 3).reshape(128, KC * JT * 128)
    wlin = Wlin.reshape(KC, 128, O).transpose(1, 0, 2).reshape(128, KC * O)
    wzr = np.ascontiguousarray(wzr, dtype=bf)
    wht = np.ascontiguousarray(wht, dtype=bf)
    wlin = np.ascontiguousarray(wlin, dtype=bf)
    ident = np.eye(128, dtype=bf)

    in_maps = []
    for i in range(NCORES):
        sl = slice(i * BL, (i + 1) * BL)
        gf = gam[:, sl, :]  # (T,BL,H)
        # gam packed: [p, t, kt*BL+b]
        gp = np.ascontiguousarray(
            gf.reshape(T, BL, KC, 128).transpose(3, 0, 2, 1).reshape(128, T, KC * BL),
            dtype=bf)

        def packA(A):
            return np.ascontiguousarray(
                A[sl].reshape(BL, JT, 128).transpose(2, 1, 0).reshape(128, JT * BL), dtype=bf)

        in_maps.append({
            "gam": gp,
            "wzr": wzr,
            "wht": wht,
            "wlin": wlin,
            "a0z": packA(Az0),
            "a0r": packA(Ar0),
            "a0h": packA(Ah0),
            "ident": ident,
        })
    return in_maps


def kernel(C, t, mask, Wz, bz, Wr, br, Wh, bh, Wgh, bgh, wgx, bgx, Wlin, blin,
           _trace=False, _trace_kwargs=None):
    C = np.asarray(C, np.float32)
    t = np.asarray(t, np.float32)
    nc = _build_program()
    in_maps = _host_prep(C, t,
                         np.asarray(Wz, np.float32), np.asarray(bz, np.float32),
                         np.asarray(Wr, np.float32), np.asarray(br, np.float32),
                         np.asarray(Wh, np.float32), np.asarray(bh, np.float32),
                         np.asarray(Wgh, np.float32), np.asarray(bgh, np.float32),
                         np.asarray(Wlin, np.float32))

    from concourse.bass_utils import run_bass_kernel_spmd
    res = run_bass_kernel_spmd(nc, in_maps, list(range(NCORES)),
                               trace=_trace, **(_trace_kwargs or {}))
    outs = [res.results[i]["out"] for i in range(NCORES)]
    full = np.concatenate(outs, axis=1).astype(np.float32)  # (T,B,O)
    full += np.asarray(blin, np.float32)[None, None, :]
    kernel._last_results = res
    return full
